# revision 67
# baseline (speedup 1.0000x reference)
"""Trainium2 Bass kernel: pre-norm transformer block (dense_transformer).

Reference (per token row x of [4096, 768]):
  h1 = LN(x; g1, b1);  qkv = h1 @ w_qkv;  attention (12 heads, dh=64, softmax)
  x1 = x + attn_out @ w_proj + b_proj
  h2 = LN(x1; g2, b2); out = x1 + gelu(h2 @ w_fc1 + b_fc1) @ w_fc2 + b_fc2

Sharding: sequence (data) parallel - each of 8 cores owns 512 tokens.  K/V of
the full sequence are exchanged with one AllGather collective per head pair;
everything else is core-local (no all-reduces at all).

Perf design on top of the bf16 baseline:
  * The softmax exp (25.2M elements/core) is the attention wall on the Act
    engine; it is now split between the Act engine (true Exp) and the Vector
    engine, which computes exp with a Schraudolph bit-trick: byte =
    score*(8*log2e*scale) + 56.5 converted to int8 IS the fp8-e4m3 encoding
    of exp(score*scale).  The softmax denominator (ones column in V_aug)
    normalizes the same quantized weights, so the error largely cancels.
  * Attention runs in fp8-e4m3: Q/K/V/P tiles are fp8.  AV uses DoubleRow
    fp8 matmuls (2 key-tiles contracted per pass, ~1.4x the bf16 rate);
    scores use PE row-tiling (two dh=64 heads in the two PE row halves).
  * QKV and proj matmuls use DoubleRow fp8 with host-interleaved weights
    ([128, nb, 2, M] layout; w' = diag(g)*w*32 in e4m3, the 32x pre-scale
    keeps the 0.02-scale weights out of the e4m3 subnormal range; the 1/1024
    descale folds into the exp scale and the proj residual step).
  * MLP (fc1/fc2) stays bf16: fp8 there costs ~1.5e-2 absmax error.
  * K/V are exchanged in fp8: half the collective bytes of bf16.

Matmul operands are bf16/fp8 (fp32 matmuls are 4x slower on the PE);
statistics, softmax sums, and the residual stream stay fp32.
"""

import os
import sys

import numpy as np

for _p in ("/opt/trn_rl_repo",):
    if os.path.isdir(_p) and _p not in sys.path:
        sys.path.insert(0, _p)

os.environ.setdefault("MYCRO_LOCAL_CACHE", "1")

import ml_dtypes  # noqa: E402

import concourse.bass as bass  # noqa: E402
import concourse.mybir as mybir  # noqa: E402
import concourse.tile as tile  # noqa: E402
from concourse import bacc  # noqa: E402

DIM = 768
N_TOK = 4096
HEADS = 12
DH = 64
HIDDEN = 4 * DIM
EPS = 1e-5
N_CORES = 8
T = N_TOK // N_CORES          # 512 local tokens per core
P = 128
CT = DIM // P                 # 6 feature tiles
KT = N_TOK // P               # 32 key tiles
LPC = T // P                  # 4 local token tiles
SCALE = DH ** -0.5
PAIRS = HEADS // 2
NB = CT // 2                  # 3 DoubleRow feature blocks of 256

WS = 32.0                     # fp8 weight pre-scale for wqkv / wproj
DS = 1.0 / (WS * WS)          # descale after a fp8xfp8 (x32 * x32) matmul
ACT_SCALE = SCALE * DS        # exp() argument scale on the Act engine
LOG2E = 1.4426950408889634
A_CONST = 8.0 * LOG2E * ACT_SCALE   # Schraudolph multiplier (e4m3 grid)
B_CONST = 56.5                      # e4m3 exponent-bias offset (+0.5 trunc)

VSTRIDE = 160                 # V_aug free stride per (j, slot); %16 == 0
HOFF = 80                     # head B column offset inside a V_aug group

F32 = mybir.dt.float32
BF16 = mybir.dt.bfloat16
F8 = mybir.dt.float8e4
I8 = mybir.dt.int8
AF = mybir.ActivationFunctionType
ALU = mybir.AluOpType
DR = mybir.MatmulPerfMode.DoubleRow

_CACHED_NC = None
LAST_RESULTS = None
GELU_FUNC = AF.Gelu  # sim_test swaps this (CoreSim lacks Gelu)


def build_nc():
    nc = bacc.Bacc(num_devices=N_CORES)

    xt = nc.declare_dram_parameter("xt", [DIM, T], F32, isOutput=False)
    wqkv = nc.declare_dram_parameter("wqkv", [P, 2 * NB * 3 * DIM], F8,
                                     isOutput=False)
    bqk = nc.declare_dram_parameter("bqk", [2 * DIM], F32, isOutput=False)
    bv = nc.declare_dram_parameter("bv", [DIM], F32, isOutput=False)
    wproj = nc.declare_dram_parameter("wproj", [P, 2 * NB * DIM], F8,
                                      isOutput=False)
    bproj = nc.declare_dram_parameter("bproj", [DIM], F32, isOutput=False)
    wfc1 = nc.declare_dram_parameter("wfc1", [DIM, HIDDEN], BF16, isOutput=False)
    bfc1 = nc.declare_dram_parameter("bfc1", [HIDDEN], F32, isOutput=False)
    wfc2 = nc.declare_dram_parameter("wfc2", [HIDDEN, DIM], BF16, isOutput=False)
    bfc2 = nc.declare_dram_parameter("bfc2", [DIM], F32, isOutput=False)
    outt = nc.declare_dram_parameter("outt", [DIM, T], F32, isOutput=True)

    with tile.TileContext(nc) as tc:
        _emit(nc, tc, xt, wqkv, bqk, bv, wproj, bproj, wfc1, bfc1, wfc2, bfc2,
              outt)
    # Bacc defers register allocation + event-semaphore splitting to
    # compile(); the PJRT exec path serializes the module as-is, so run
    # them now.
    nc.finalize()
    return nc


def _emit(nc, tc, xt, wqkv, bqk, bv, wproj, bproj, wfc1, bfc1, wfc2, bfc2, outt):
    from contextlib import ExitStack

    top = ExitStack()

    def pool(name, bufs, space="SBUF", stack=None):
        return (stack or top).enter_context(
            tc.tile_pool(name=name, bufs=bufs, space=space))

    # ---- long-lived SBUF pools ----
    const = pool("const", 1)
    xpool = pool("x", 1)               # x^T fp32, lives to the proj residual
    hpool = pool("h", 1)               # LN temporaries + normalized output
    qkpool = pool("qk", 1)             # Q^T fp8 (lives through phase B)
    vpool = pool("vloc", 1)            # local V token-major fp8
    kpair = pool("kpair", 2)           # streamed gathered K^T [128, 4096] fp8
    vpair = pool("vpair", 2)           # streamed gathered V_aug fp8
    aopool = pool("ao", 1)             # attention out^T fp8 [128, CT, T]
    x1pool = pool("x1", 1)             # post-attention residual fp32
    gpool = pool("g", 12)              # gelu activations bf16
    opool = pool("o", 3)               # output fp32 staging
    wpool = pool("w", 7)               # bf16 weight bands, one shared tag
    wppool = pool("wp", 1)             # fp8 DoubleRow proj weights
    stat = pool("stat", 1)             # small [1, T] statistics
    ptpool = pool("pt", 4)             # P = exp(scores) fp8 [128, 2, 1024]
    dram = pool("dram", 1, space="DRAM")

    # ---- constants / bias vectors ----
    ones_stat = const.tile([P, 1], BF16)
    nc.vector.memset(ones_stat[:], 1.0)
    ones_row = const.tile([1, P], BF16)
    nc.vector.memset(ones_row[:], 1.0)
    zero_bias = const.tile([P, 1], F32)
    nc.vector.memset(zero_bias[:], 0.0)
    eps_tile = const.tile([1, 1], F32)
    nc.vector.memset(eps_tile[:], EPS)

    bqk_sb = const.tile([P, 2 * DIM // P], F32)
    nc.sync.dma_start(bqk_sb[:], bqk.rearrange("(t p) -> p t", p=P))
    bv_sb = const.tile([1, DIM], F32)
    nc.sync.dma_start(bv_sb[:], bv[None, :])
    bproj_sb = const.tile([P, CT], F32)
    nc.sync.dma_start(bproj_sb[:], bproj.rearrange("(t p) -> p t", p=P))
    bfc1_sb = const.tile([P, HIDDEN // P], F32)
    nc.sync.dma_start(bfc1_sb[:], bfc1.rearrange("(t p) -> p t", p=P))
    bfc2_sb = const.tile([P, CT], F32)
    nc.sync.dma_start(bfc2_sb[:], bfc2.rearrange("(t p) -> p t", p=P))
    bv_bc = const.tile([P, DIM], F32)

    # ---- x^T ----  (column-split DMAs: one queue per chunk halves latency)
    x_sb = [xpool.tile([P, T], F32, name=f"x{t}") for t in range(CT)]
    for t in range(CT):
        for half in range(2):
            nc.sync.dma_start(
                x_sb[t][:, half * (T // 2):(half + 1) * (T // 2)],
                xt[t * P:(t + 1) * P, half * (T // 2):(half + 1) * (T // 2)])

    def wband(kt, src, width):
        b = wpool.tile([P, HIDDEN], BF16, tag="wband", name=f"wb{kt}")
        bb = b[:, :width]
        nc.sync.dma_start(bb, src[kt * P:(kt + 1) * P, :])
        return bb

    # ---- layernorm: (x - mean) * rsqrt(var + eps); out fp8 big tile or
    # ---- per-tile bf16 list.  Split so callers can interleave the stats
    # ---- pass with the producer of src_tiles. ----
    def ln_stats(src_tile, t, nm, s_ps, sq_ps):
        xb = hpool.tile([P, T], BF16, tag="lnxb", bufs=2, name=f"{nm}xb{t}")
        nc.vector.tensor_copy(xb[:], src_tile[:])
        xsq = hpool.tile([P, T], BF16, tag="lnxsq", bufs=2, name=f"{nm}sq{t}")
        nc.vector.tensor_mul(xsq[:], xb[:], xb[:])
        nc.tensor.matmul(s_ps[:], ones_stat[:], xb[:],
                         start=(t == 0), stop=(t == CT - 1))
        nc.tensor.matmul(sq_ps[:], ones_stat[:], xsq[:],
                         start=(t == 0), stop=(t == CT - 1))

    def layernorm(src_tiles, nm, stps, bcps, big_out=None):
        s_ps = stps.tile([1, T], F32, tag="s")
        sq_ps = stps.tile([1, T], F32, tag="sq")
        for t in range(CT):
            ln_stats(src_tiles[t], t, nm, s_ps, sq_ps)
        return ln_finish(src_tiles, nm, s_ps, sq_ps, bcps, big_out)

    def ln_finish(src_tiles, nm, s_ps, sq_ps, bcps, big_out=None):
        ssum = stat.tile([1, T], F32, tag="lnf", bufs=5, name=f"{nm}sum")
        nc.vector.tensor_copy(ssum[:], s_ps[:])
        t1 = stat.tile([1, T], F32, tag="lnf", bufs=5, name=f"{nm}t1")
        nc.vector.scalar_tensor_tensor(t1[:], ssum[:], 1.0 / DIM, ssum[:],
                                       ALU.mult, ALU.mult)
        t2 = stat.tile([1, T], F32, tag="lnf", bufs=5, name=f"{nm}t2")
        nc.vector.tensor_sub(t2[:], sq_ps[:], t1[:])
        sdev = stat.tile([1, T], F32, tag="lnf", bufs=5, name=f"{nm}sdev")
        nc.scalar.activation(sdev[:], t2[:], AF.Sqrt,
                             bias=eps_tile[:], scale=1.0 / DIM)
        rstd = stat.tile([1, T], F32, tag="lnf", bufs=5, name=f"{nm}rstd")
        nc.vector.reciprocal(rstd[:], sdev[:])
        rstd_b = stat.tile([1, T], BF16, tag="lnb", bufs=2, name=f"{nm}rstdb")
        nc.vector.tensor_copy(rstd_b[:], rstd[:])
        mrs_b = stat.tile([1, T], BF16, tag="lnb", bufs=2, name=f"{nm}mrsb")
        nc.vector.scalar_tensor_tensor(mrs_b[:], ssum[:], 1.0 / DIM, rstd[:],
                                       ALU.mult, ALU.mult)
        rstd_ps = bcps.tile([P, T], F32, tag="bc")
        nc.tensor.matmul(rstd_ps[:], ones_row[:], rstd_b[:], start=True, stop=True)
        mrs_ps = bcps.tile([P, T], F32, tag="bc")
        nc.tensor.matmul(mrs_ps[:], ones_row[:], mrs_b[:], start=True, stop=True)
        out = []
        for t in range(CT):
            tmp = hpool.tile([P, T], F32, tag="lntmp", bufs=2, name=f"{nm}tm{t}")
            nc.vector.tensor_mul(tmp[:], src_tiles[t][:], rstd_ps[:])
            if big_out is not None:
                nc.vector.tensor_sub(big_out[:, t, :], tmp[:], mrs_ps[:])
            else:
                ht = hpool.tile([P, T], BF16, tag="lnout", bufs=CT,
                                name=f"{nm}o{t}")
                nc.vector.tensor_sub(ht[:], tmp[:], mrs_ps[:])
                out.append(ht)
        return out

    # ======================= phase A: LN1, QKV, V-local ======================
    pA = ExitStack()
    stpsA = pool("stpsA", 1, space="PSUM", stack=pA)
    bcpsA = pool("bcpsA", 2, space="PSUM", stack=pA)
    mmpsA = pool("mmpsA", 2, space="PSUM", stack=pA)
    vps = pool("vps", 1, space="PSUM", stack=pA)
    wqpool = pool("wq", 1, stack=pA)   # fp8 DoubleRow qkv weights (phase A)
    klpool = pool("kloc", 1, stack=pA)  # local K^T fp8 (gathered, then dead)

    # ---- fp8 DoubleRow qkv weights: [128, NB, 2, 3*DIM] ----
    # column-split into 3 chunks per block so the loads spread over queues
    wq_sb = wqpool.tile([P, NB, 2, 3 * DIM], F8, name="wq")
    for b in range(NB):
        src = wqkv[:, b * 2 * 3 * DIM:(b + 1) * 2 * 3 * DIM].rearrange(
            "p (i m) -> p i m", i=2)
        for ch in range(3):
            nc.sync.dma_start(
                wq_sb[:, b, :, ch * DIM:(ch + 1) * DIM],
                src[:, :, ch * DIM:(ch + 1) * DIM])

    # broadcast bv across partitions (once)
    bv_b = const.tile([1, DIM], BF16)
    nc.vector.tensor_copy(bv_b[:], bv_sb[:])
    bv_ps = vps.tile([P, DIM], F32, tag="vps")
    nc.tensor.matmul(bv_ps[:, 0:512], ones_row[:], bv_b[:, 0:512],
                     start=True, stop=True)
    nc.tensor.matmul(bv_ps[:, 512:DIM], ones_row[:], bv_b[:, 512:DIM],
                     start=True, stop=True)
    nc.vector.tensor_copy(bv_bc[:], bv_ps[:])

    h1 = hpool.tile([P, CT, T], F8, name="h1big")
    layernorm(x_sb, "h1", stpsA, bcpsA, big_out=h1)

    qk_sb = [qkpool.tile([P, T], F8, name=f"qk{m}") if m < CT else
             klpool.tile([P, T], F8, name=f"qk{m}") for m in range(2 * CT)]

    def qk_proj(m):
        ps = mmpsA.tile([P, T], F32, tag="mm")
        for b in range(NB):
            nc.tensor.matmul(ps[:], wq_sb[:, b, :, m * P:(m + 1) * P],
                             h1[:, 2 * b:2 * b + 2, :],
                             start=(b == 0), stop=(b == NB - 1), perf_mode=DR)
        nc.vector.tensor_scalar_add(qk_sb[m][:], ps[:], bqk_sb[:, m:m + 1])

    KSZ = P * T                      # 65536 elems: this pair's K^T shard
    VSZ = T * 2 * DH                 # 65536 elems: this pair's V shard
    PRSZ = KSZ + VSZ
    kv_out = []

    def gather(pr):
        kv_in_pr = dram.tile([PRSZ], F8, name=f"kvi{pr}")
        kv_out_pr = dram.tile([N_CORES * PRSZ], F8, name=f"kvo{pr}",
                              addr_space="Shared")
        nc.sync.dma_start(kv_in_pr[0:KSZ], qk_sb[CT + pr][:])
        for mt in range(LPC):
            nc.sync.dma_start(
                kv_in_pr[KSZ + mt * P * 2 * DH:KSZ + (mt + 1) * P * 2 * DH],
                v_sb[mt][:, 2 * pr * DH:(2 * pr + 2) * DH])
        nc.gpsimd.collective_compute(
            "AllGather", ALU.bypass,
            replica_groups=[list(range(N_CORES))],
            ins=[kv_in_pr[:]], outs=[kv_out_pr[:]])
        kv_out.append(kv_out_pr)

    v_sb = [vpool.tile([P, DIM], F8, name=f"v{mt}") for mt in range(LPC)]

    def v_proj(n0, nw):
        for mt in range(LPC):
            ps = vps.tile([P, 512], F32, tag="vps")
            for b in range(NB):
                nc.tensor.matmul(
                    ps[:, 0:nw],
                    h1[:, 2 * b:2 * b + 2, mt * P:(mt + 1) * P],
                    wq_sb[:, b, :, 2 * DIM + n0:2 * DIM + n0 + nw],
                    start=(b == 0), stop=(b == NB - 1), perf_mode=DR)
            nc.vector.scalar_tensor_tensor(v_sb[mt][:, n0:n0 + nw], ps[:, 0:nw],
                                           1.0, bv_bc[:, n0:n0 + nw],
                                           ALU.mult, ALU.add)

    # pair 0's K tile and V columns first so its gather launches ~30us
    # earlier; the rest of K/V (and gathers 1-5) follow, then Q
    qk_proj(CT)
    v_proj(0, P)
    gather(0)
    for m in range(CT + 1, 2 * CT):
        qk_proj(m)
    v_proj(P, 384)
    v_proj(512, 256)
    for pr in range(1, PAIRS):
        gather(pr)

    # Q projections run while the gathers are in flight
    for m in range(CT):
        qk_proj(m)

    pA.close()

    # proj weights: DMA emitted at pair 1 (below) so pair 0's gathered K/V
    # loads aren't queued behind it
    wp_sb = wppool.tile([P, NB, 2, DIM], F8, name="wp")

    def load_wproj():
        for b in range(NB):
            nc.sync.dma_start(
                wp_sb[:, b, :, :],
                wproj[:, b * 2 * DIM:(b + 1) * 2 * DIM].rearrange(
                    "p (i m) -> p i m", i=2))

    def load_kpair(pr):
        kt_ = kpair.tile([P, N_TOK], F8, tag="kp", name=f"kp{pr}")
        for c in range(N_CORES):
            src = kv_out[pr][c * PRSZ:c * PRSZ + KSZ]
            nc.sync.dma_start(kt_[:, c * T:(c + 1) * T],
                              src.rearrange("(p q) -> p q", q=T))
        return kt_

    def load_vpair(pr):
        # V_aug layout: [p, j(16), slot(2), VSTRIDE] where the VSTRIDE group
        # holds head A cols 0:64, A-ones at 64, head B cols 80:144, B-ones
        # at 144.  Key of (p, j, slot) = 128*(2j + slot) + p.
        vt = vpair.tile([P, KT // 2, 2, VSTRIDE], F8, tag="vp", name=f"vp{pr}")
        for c in range(N_CORES):
            src = kv_out[pr][c * PRSZ + KSZ:c * PRSZ + KSZ + VSZ]
            src4 = src.rearrange("(jl i p f) -> p jl i f", i=2, p=P, f=2 * DH)
            for h in range(2):
                nc.sync.dma_start(
                    vt[:, 2 * c:2 * c + 2, :, h * HOFF:h * HOFF + DH],
                    src4[:, :, :, h * DH:(h + 1) * DH])
        nc.gpsimd.memset(vt[:, :, :, DH:DH + 1], 1.0)
        nc.gpsimd.memset(vt[:, :, :, HOFF + DH:HOFF + DH + 1], 1.0)
        return vt

    # ======================= phase B: attention ==============================
    pB = ExitStack()
    scps = pool("scps", 3, space="PSUM", stack=pB)
    accps = pool("accps", 2, space="PSUM", stack=pB)

    JT = KT // 2
    ao_big = aopool.tile([P, CT, T], F8, name="aobig")
    pending = None  # previous pair's deferred normalization
    for pr in range(PAIRS):
        q_tile = qk_sb[pr]
        k_tile = load_kpair(pr)
        v_tile = load_vpair(pr)
        if pr == 1:
            load_wproj()
        # previous pair's normalization BEFORE reusing its acc psum slots
        # (accps bufs=2: acc(pr) recycles acc(pr-1)'s banks)
        if pending is not None:
            pending()
        acc_a = accps.tile([P, T], F32, tag="acc", name=f"acca{pr}")
        acc_b = accps.tile([P, T], F32, tag="acc", name=f"accb{pr}")

        def scores(kt, pr=pr, k_tile=k_tile, q_tile=q_tile):
            sc = scps.tile([P, 2 * T], F32, tag="sc", name=f"sc{pr}_{kt}")
            nc.tensor.matmul(sc[:, 0:T], k_tile[0:DH, kt * P:(kt + 1) * P],
                             q_tile[0:DH, :], start=True, stop=True)
            nc.tensor.matmul(sc[:, T:2 * T], k_tile[DH:P, kt * P:(kt + 1) * P],
                             q_tile[DH:P, :], start=True, stop=True)
            return sc

        def av(j, pt, acc_a=acc_a, acc_b=acc_b, v_tile=v_tile):
            nc.tensor.matmul(acc_a[0:DH + 1, :], v_tile[:, j, :, 0:DH + 1],
                             pt[:, :, 0:T], start=(j == 0), stop=(j == JT - 1),
                             perf_mode=DR)
            nc.tensor.matmul(acc_b[0:DH + 1, :],
                             v_tile[:, j, :, HOFF:HOFF + DH + 1],
                             pt[:, :, T:2 * T], start=(j == 0),
                             stop=(j == JT - 1), perf_mode=DR)

        prev_pt = None
        for j in range(JT):
            # the DVE (slot 1) is the rate-limiting exp engine: compute its
            # score tile FIRST each round so it never waits on the PE
            sc1 = scores(2 * j + 1)
            pt = ptpool.tile([P, 2, 2 * T], F8, tag="pt", name=f"pt{pr}_{j}")
            if j == JT - 1:
                # 17/15 Act/DVE split
                nc.scalar.activation(pt[:, 1, :], sc1[:], AF.Exp,
                                     bias=zero_bias[:], scale=ACT_SCALE)
            else:
                nc.vector.tensor_scalar(pt[:, 1, :].bitcast(I8), sc1[:],
                                        A_CONST, B_CONST, ALU.mult, ALU.add)
            sc0 = scores(2 * j)
            nc.scalar.activation(pt[:, 0, :], sc0[:], AF.Exp,
                                 bias=zero_bias[:], scale=ACT_SCALE)
            if j > 0:
                av(j - 1, prev_pt)
            prev_pt = pt
        av(JT - 1, prev_pt)

        def mk_finish(pr, acc_a, acc_b):
            def fin():
                # softmax denominators: Act stages sumexp to SBUF, GpSimd
                # computes 1/x (pow -1) + the partition broadcast; the busy
                # Vector engine only does the final normalize multiplies
                for half, acc in ((0, acc_a), (1, acc_b)):
                    rec_b = stat.tile([1, T], BF16, tag="recb", bufs=2,
                                      name=f"rb{pr}{half}")
                    with nc.allow_low_precision(
                            "softmax denominator broadcast is bf16 anyway"):
                        nc.vector.reciprocal(rec_b[:], acc[DH:DH + 1, :])
                    bc_sb = stat.tile([DH, T], BF16, tag="bcsb", bufs=2,
                                      name=f"bs{pr}{half}")
                    nc.gpsimd.partition_broadcast(bc_sb[:], rec_b[:])
                    nc.vector.tensor_mul(
                        ao_big[half * DH:(half + 1) * DH, pr, :],
                        acc[0:DH, :], bc_sb[:])
            return fin

        pending = mk_finish(pr, acc_a, acc_b)

    pending()
    pB.close()

    # ======================= phase C1: proj + residual + LN2 =================
    pC1 = ExitStack()
    stpsC = pool("stpsC", 1, space="PSUM", stack=pC1)
    bcpsC = pool("bcpsC", 2, space="PSUM", stack=pC1)
    mmpsC = pool("mmpsC", 2, space="PSUM", stack=pC1)

    x1_sb = [x1pool.tile([P, T], F32, name=f"x1_{m}") for m in range(CT)]
    s2ps = stpsC.tile([1, T], F32, tag="s")
    sq2ps = stpsC.tile([1, T], F32, tag="sq")
    for m in range(CT):
        ps = mmpsC.tile([P, T], F32, tag="mm")
        for b in range(NB):
            nc.tensor.matmul(ps[:], wp_sb[:, b, :, m * P:(m + 1) * P],
                             ao_big[:, 2 * b:2 * b + 2, :],
                             start=(b == 0), stop=(b == NB - 1), perf_mode=DR)
        tmp = hpool.tile([P, T], F32, tag="lntmp", bufs=2, name=f"prt{m}")
        nc.vector.scalar_tensor_tensor(tmp[:], ps[:], DS, x_sb[m][:],
                                       ALU.mult, ALU.add)
        nc.vector.tensor_scalar_add(x1_sb[m][:], tmp[:], bproj_sb[:, m:m + 1])
        # LN2 stats interleave with the proj loop (x1 tile m just completed)
        ln_stats(x1_sb[m], m, "h2", s2ps, sq2ps)

    h2 = ln_finish(x1_sb, "h2", s2ps, sq2ps, bcpsC)
    pC1.close()

    # ======================= phase C2: MLP ===================================
    pC2 = ExitStack()
    mmpsM = pool("mmpsM", 2, space="PSUM", stack=pC2)
    fc2ps = pool("fc2ps", CT, space="PSUM", stack=pC2)

    fc1_bands = [wband(kt, wfc1, HIDDEN) for kt in range(CT)]
    g_sb = [gpool.tile([P, T], BF16, tag="g", name=f"g{m}")
            for m in range(HIDDEN // P)]
    o_ps = [fc2ps.tile([P, T], F32, tag="oacc", name=f"ops{m}") for m in range(CT)]

    def fc2_step(kt):
        band = wband(kt, wfc2, DIM)
        for m in range(CT):
            nc.tensor.matmul(o_ps[m][:], band[:, m * P:(m + 1) * P], g_sb[kt][:],
                             start=(kt == 0), stop=(kt == HIDDEN // P - 1))

    # fc2's kt-accumulation interleaves with fc1's m-loop: fc2(kt) only
    # needs g_sb[kt], so the PE stream never waits on a gelu
    for m in range(HIDDEN // P):
        ps = mmpsM.tile([P, T], F32, tag="mm")
        for kt in range(CT):
            nc.tensor.matmul(ps[:], fc1_bands[kt][:, m * P:(m + 1) * P],
                             h2[kt][:], start=(kt == 0), stop=(kt == CT - 1))
        nc.scalar.activation(g_sb[m][:], ps[:], GELU_FUNC,
                             bias=bfc1_sb[:, m:m + 1], scale=1.0)
        if m >= 2:
            fc2_step(m - 2)
    fc2_step(HIDDEN // P - 2)
    fc2_step(HIDDEN // P - 1)
    for m in range(CT):
        ot = opool.tile([P, T], F32, tag="ot", name=f"ot{m}")
        nc.vector.scalar_tensor_tensor(ot[:], o_ps[m][:], bfc2_sb[:, m:m + 1],
                                       x1_sb[m][:], ALU.add, ALU.add)
        nc.sync.dma_start(outt[m * P:(m + 1) * P, :], ot[:])
    pC2.close()
    top.close()


def _interleave_dr(w):
    """[DIM_in, M] -> DoubleRow stationary layout [128, NB*2*M] where
    weight row (p, b, i) holds input feature 256*b + 128*i + p."""
    din, m = w.shape
    nb = din // 256
    w4 = w.reshape(nb, 2, P, m)          # [b, i, p, m]
    w4 = w4.transpose(2, 0, 1, 3)        # [p, b, i, m]
    return np.ascontiguousarray(w4.reshape(P, nb * 2 * m))


def _prepare_in_maps(inputs):
    x = np.asarray(inputs["x"], np.float32)
    g1 = np.asarray(inputs["g1"], np.float32)
    b1 = np.asarray(inputs["b1"], np.float32)
    g2 = np.asarray(inputs["g2"], np.float32)
    b2 = np.asarray(inputs["b2"], np.float32)
    w_qkv = np.asarray(inputs["w_qkv"], np.float32)
    w_proj = np.asarray(inputs["w_proj"], np.float32)
    b_proj = np.asarray(inputs["b_proj"], np.float32)
    w_fc1 = np.asarray(inputs["w_fc1"], np.float32)
    b_fc1 = np.asarray(inputs["b_fc1"], np.float32)
    w_fc2 = np.asarray(inputs["w_fc2"], np.float32)
    b_fc2 = np.asarray(inputs["b_fc2"], np.float32)

    bf = ml_dtypes.bfloat16
    f8 = mybir.dt.np(F8)
    wqkv_eff = _interleave_dr(g1[:, None] * w_qkv * WS).astype(f8)
    bqkv_eff = (b1 @ w_qkv).astype(np.float32) * WS
    wproj_eff = _interleave_dr(w_proj * WS).astype(f8)
    wfc1_eff = (g2[:, None] * w_fc1).astype(bf)
    bfc1_eff = (b_fc1 + b2 @ w_fc1).astype(np.float32)

    shared = {
        "wqkv": wqkv_eff,
        "bqk": np.ascontiguousarray(bqkv_eff[:2 * DIM]),
        "bv": np.ascontiguousarray(bqkv_eff[2 * DIM:]),
        "wproj": wproj_eff,
        "bproj": b_proj,
        "wfc1": wfc1_eff,
        "bfc1": bfc1_eff,
        "wfc2": w_fc2.astype(bf),
        "bfc2": b_fc2,
    }
    in_maps = []
    for c in range(N_CORES):
        xs = np.ascontiguousarray(x[0, c * T:(c + 1) * T, :].T)
        in_maps.append({"xt": xs, **shared})
    return in_maps


def _install_ntff_hook():
    """The agent image's antenv lacks axon_hooks; synthesize it so
    BASS_TRACE=1 profiling works (and its absence never crashes)."""
    import types
    try:
        from antenv.axon_hooks import get_axon_ntff_profile_hook  # noqa: F401
        return
    except ImportError:
        pass
    try:
        import antenv
        mod = types.ModuleType("antenv.axon_hooks")
        _h = [None]
        mod.set_axon_ntff_profile_hook = lambda h: _h.__setitem__(0, h)
        mod.get_axon_ntff_profile_hook = lambda: _h[0]
        sys.modules["antenv.axon_hooks"] = mod
        antenv.axon_hooks = mod
        try:
            from trn_agent_boot.trn_boot import _ntff_profile_via_ctypes
            so = "/opt/axon/libaxon_pjrt.so"
            if os.path.exists(so):
                mod.set_axon_ntff_profile_hook(_ntff_profile_via_ctypes(so))
        except Exception:
            pass
    except Exception:
        pass


def kernel(**inputs):
    global _CACHED_NC, LAST_RESULTS
    from concourse.bass_utils import run_bass_kernel_spmd

    _install_ntff_hook()

    if _CACHED_NC is None:
        _CACHED_NC = build_nc()
    nc = _CACHED_NC
    in_maps = _prepare_in_maps(inputs)
    res = run_bass_kernel_spmd(nc, in_maps, list(range(N_CORES)))
    LAST_RESULTS = res
    out = np.empty((1, N_TOK, DIM), np.float32)
    for c in range(N_CORES):
        out[0, c * T:(c + 1) * T, :] = res.results[c]["outt"].T
    return out


# revision 68
# speedup vs baseline: 1.1411x; 1.1411x over previous
"""Trainium2 Bass kernel: pre-norm transformer block (dense_transformer).

Reference (per token row x of [4096, 768]):
  h1 = LN(x; g1, b1);  qkv = h1 @ w_qkv;  attention (12 heads, dh=64, softmax)
  x1 = x + attn_out @ w_proj + b_proj
  h2 = LN(x1; g2, b2); out = x1 + gelu(h2 @ w_fc1 + b_fc1) @ w_fc2 + b_fc2

Sharding: sequence (data) parallel - each of 8 cores owns 512 tokens.  K/V of
the full sequence are exchanged with one AllGather collective per head pair;
everything else is core-local (no all-reduces at all).

Perf design on top of the bf16 baseline:
  * The softmax exp (25.2M elements/core) is the attention wall on the Act
    engine; it is now split between the Act engine (true Exp) and the Vector
    engine, which computes exp with a Schraudolph bit-trick: byte =
    score*(8*log2e*scale) + 56.5 converted to int8 IS the fp8-e4m3 encoding
    of exp(score*scale).  The softmax denominator (ones column in V_aug)
    normalizes the same quantized weights, so the error largely cancels.
  * Attention runs in fp8-e4m3: Q/K/V/P tiles are fp8.  AV uses DoubleRow
    fp8 matmuls (2 key-tiles contracted per pass, ~1.4x the bf16 rate);
    scores use PE row-tiling (two dh=64 heads in the two PE row halves).
  * QKV and proj matmuls use DoubleRow fp8 with host-interleaved weights
    ([128, nb, 2, M] layout; w' = diag(g)*w*32 in e4m3, the 32x pre-scale
    keeps the 0.02-scale weights out of the e4m3 subnormal range; the 1/1024
    descale folds into the exp scale and the proj residual step).
  * MLP (fc1/fc2) stays bf16: fp8 there costs ~1.5e-2 absmax error.
  * K/V are exchanged in fp8: half the collective bytes of bf16.

Matmul operands are bf16/fp8 (fp32 matmuls are 4x slower on the PE);
statistics, softmax sums, and the residual stream stay fp32.
"""

import os
import sys

import numpy as np

for _p in ("/opt/trn_rl_repo",):
    if os.path.isdir(_p) and _p not in sys.path:
        sys.path.insert(0, _p)

os.environ.setdefault("MYCRO_LOCAL_CACHE", "1")

import ml_dtypes  # noqa: E402

import concourse.bass as bass  # noqa: E402
import concourse.mybir as mybir  # noqa: E402
import concourse.tile as tile  # noqa: E402
from concourse import bacc  # noqa: E402

DIM = 768
N_TOK = 4096
HEADS = 12
DH = 64
HIDDEN = 4 * DIM
EPS = 1e-5
N_CORES = 8
T = N_TOK // N_CORES          # 512 local tokens per core
P = 128
CT = DIM // P                 # 6 feature tiles
KT = N_TOK // P               # 32 key tiles
LPC = T // P                  # 4 local token tiles
SCALE = DH ** -0.5
PAIRS = HEADS // 2
NB = CT // 2                  # 3 DoubleRow feature blocks of 256

WS = 32.0                     # fp8 weight pre-scale for wqkv / wproj
DS = 1.0 / (WS * WS)          # descale after a fp8xfp8 (x32 * x32) matmul
ACT_SCALE = SCALE * DS        # exp() argument scale on the Act engine
LOG2E = 1.4426950408889634
A_CONST = 8.0 * LOG2E * ACT_SCALE   # Schraudolph multiplier (e4m3 grid)
B_CONST = 56.5                      # e4m3 exponent-bias offset (+0.5 trunc)

VSTRIDE = 160                 # V_aug free stride per (j, slot); %16 == 0
HOFF = 80                     # head B column offset inside a V_aug group

F32 = mybir.dt.float32
BF16 = mybir.dt.bfloat16
F8 = mybir.dt.float8e4
I8 = mybir.dt.int8
AF = mybir.ActivationFunctionType
ALU = mybir.AluOpType
DR = mybir.MatmulPerfMode.DoubleRow

_CACHED_NC = None
LAST_RESULTS = None
GELU_FUNC = AF.Gelu  # sim_test swaps this (CoreSim lacks Gelu)


def build_nc():
    nc = bacc.Bacc(num_devices=N_CORES)

    xt = nc.declare_dram_parameter("xt", [DIM, T], F32, isOutput=False)
    wqkv = nc.declare_dram_parameter("wqkv", [P, 2 * NB * 3 * DIM], F8,
                                     isOutput=False)
    bqk = nc.declare_dram_parameter("bqk", [2 * DIM], F32, isOutput=False)
    bv = nc.declare_dram_parameter("bv", [DIM], F32, isOutput=False)
    wproj = nc.declare_dram_parameter("wproj", [P, 2 * NB * DIM], F8,
                                      isOutput=False)
    bproj = nc.declare_dram_parameter("bproj", [DIM], F32, isOutput=False)
    wfc1 = nc.declare_dram_parameter("wfc1", [DIM, HIDDEN], BF16, isOutput=False)
    bfc1 = nc.declare_dram_parameter("bfc1", [HIDDEN], F32, isOutput=False)
    wfc2 = nc.declare_dram_parameter("wfc2", [HIDDEN, DIM], BF16, isOutput=False)
    bfc2 = nc.declare_dram_parameter("bfc2", [DIM], F32, isOutput=False)
    outt = nc.declare_dram_parameter("outt", [DIM, T], F32, isOutput=True)

    with tile.TileContext(nc) as tc:
        _emit(nc, tc, xt, wqkv, bqk, bv, wproj, bproj, wfc1, bfc1, wfc2, bfc2,
              outt)
    # Bacc defers register allocation + event-semaphore splitting to
    # compile(); the PJRT exec path serializes the module as-is, so run
    # them now.
    nc.finalize()
    return nc


def _emit(nc, tc, xt, wqkv, bqk, bv, wproj, bproj, wfc1, bfc1, wfc2, bfc2, outt):
    from contextlib import ExitStack

    top = ExitStack()

    def pool(name, bufs, space="SBUF", stack=None):
        return (stack or top).enter_context(
            tc.tile_pool(name=name, bufs=bufs, space=space))

    # ---- long-lived SBUF pools ----
    const = pool("const", 1)
    xpool = pool("x", 1)               # x^T fp32, lives to the proj residual
    hpool = pool("h", 1)               # LN temporaries + normalized output
    qkpool = pool("qk", 1)             # Q^T fp8 (lives through phase B)
    vpool = pool("vloc", 1)            # local V token-major fp8
    kpair = pool("kpair", 2)           # streamed gathered K^T [128, 4096] fp8
    vpair = pool("vpair", 2)           # streamed gathered V_aug fp8
    aopool = pool("ao", 1)             # attention out^T fp8 [128, CT, T]
    x1pool = pool("x1", 1)             # post-attention residual fp32
    gpool = pool("g", 12)              # gelu activations bf16
    opool = pool("o", 3)               # output fp32 staging
    wpool = pool("w", 7)               # bf16 weight bands, one shared tag
    wppool = pool("wp", 1)             # fp8 DoubleRow proj weights
    stat = pool("stat", 1)             # small [1, T] statistics
    ptpool = pool("pt", 4)             # P = exp(scores) fp8 [128, 2, 1024]
    dram = pool("dram", 1, space="DRAM")

    # ---- constants / bias vectors ----
    ones_stat = const.tile([P, 1], BF16)
    nc.vector.memset(ones_stat[:], 1.0)
    ones_row = const.tile([1, P], BF16)
    nc.vector.memset(ones_row[:], 1.0)
    zero_bias = const.tile([P, 1], F32)
    nc.vector.memset(zero_bias[:], 0.0)
    eps_tile = const.tile([1, 1], F32)
    nc.vector.memset(eps_tile[:], EPS)

    bqk_sb = const.tile([P, 2 * DIM // P], F32)
    nc.sync.dma_start(bqk_sb[:], bqk.rearrange("(t p) -> p t", p=P))
    bv_sb = const.tile([1, DIM], F32)
    nc.sync.dma_start(bv_sb[:], bv[None, :])
    bproj_sb = const.tile([P, CT], F32)
    nc.sync.dma_start(bproj_sb[:], bproj.rearrange("(t p) -> p t", p=P))
    bfc1_sb = const.tile([P, HIDDEN // P], F32)
    nc.sync.dma_start(bfc1_sb[:], bfc1.rearrange("(t p) -> p t", p=P))
    bfc2_sb = const.tile([P, CT], F32)
    nc.sync.dma_start(bfc2_sb[:], bfc2.rearrange("(t p) -> p t", p=P))
    bv_bc = const.tile([P, DIM], F32)

    # ---- x^T ----  (column-split DMAs: one queue per chunk halves latency)
    x_sb = [xpool.tile([P, T], F32, name=f"x{t}") for t in range(CT)]
    for t in range(CT):
        for half in range(2):
            nc.sync.dma_start(
                x_sb[t][:, half * (T // 2):(half + 1) * (T // 2)],
                xt[t * P:(t + 1) * P, half * (T // 2):(half + 1) * (T // 2)])

    def wband(kt, src, width):
        b = wpool.tile([P, HIDDEN], BF16, tag="wband", name=f"wb{kt}")
        bb = b[:, :width]
        nc.sync.dma_start(bb, src[kt * P:(kt + 1) * P, :])
        return bb

    # ---- layernorm: (x - mean) * rsqrt(var + eps); out fp8 big tile or
    # ---- per-tile bf16 list.  Split so callers can interleave the stats
    # ---- pass with the producer of src_tiles. ----
    def ln_stats(src_tile, t, nm, s_ps, sq_ps):
        xb = hpool.tile([P, T], BF16, tag="lnxb", bufs=2, name=f"{nm}xb{t}")
        nc.vector.tensor_copy(xb[:], src_tile[:])
        xsq = hpool.tile([P, T], BF16, tag="lnxsq", bufs=2, name=f"{nm}sq{t}")
        nc.vector.tensor_mul(xsq[:], xb[:], xb[:])
        nc.tensor.matmul(s_ps[:], ones_stat[:], xb[:],
                         start=(t == 0), stop=(t == CT - 1))
        nc.tensor.matmul(sq_ps[:], ones_stat[:], xsq[:],
                         start=(t == 0), stop=(t == CT - 1))

    def layernorm(src_tiles, nm, stps, bcps, big_out=None):
        s_ps = stps.tile([1, T], F32, tag="s")
        sq_ps = stps.tile([1, T], F32, tag="sq")
        for t in range(CT):
            ln_stats(src_tiles[t], t, nm, s_ps, sq_ps)
        return ln_finish(src_tiles, nm, s_ps, sq_ps, bcps, big_out)

    def ln_finish(src_tiles, nm, s_ps, sq_ps, bcps, big_out=None):
        ssum = stat.tile([1, T], F32, tag="lnf", bufs=5, name=f"{nm}sum")
        nc.vector.tensor_copy(ssum[:], s_ps[:])
        t1 = stat.tile([1, T], F32, tag="lnf", bufs=5, name=f"{nm}t1")
        nc.vector.scalar_tensor_tensor(t1[:], ssum[:], 1.0 / DIM, ssum[:],
                                       ALU.mult, ALU.mult)
        t2 = stat.tile([1, T], F32, tag="lnf", bufs=5, name=f"{nm}t2")
        nc.vector.tensor_sub(t2[:], sq_ps[:], t1[:])
        sdev = stat.tile([1, T], F32, tag="lnf", bufs=5, name=f"{nm}sdev")
        nc.scalar.activation(sdev[:], t2[:], AF.Sqrt,
                             bias=eps_tile[:], scale=1.0 / DIM)
        rstd = stat.tile([1, T], F32, tag="lnf", bufs=5, name=f"{nm}rstd")
        nc.vector.reciprocal(rstd[:], sdev[:])
        rstd_b = stat.tile([1, T], BF16, tag="lnb", bufs=2, name=f"{nm}rstdb")
        nc.vector.tensor_copy(rstd_b[:], rstd[:])
        mrs_b = stat.tile([1, T], BF16, tag="lnb", bufs=2, name=f"{nm}mrsb")
        nc.vector.scalar_tensor_tensor(mrs_b[:], ssum[:], 1.0 / DIM, rstd[:],
                                       ALU.mult, ALU.mult)
        rstd_ps = bcps.tile([P, T], F32, tag="bc")
        nc.tensor.matmul(rstd_ps[:], ones_row[:], rstd_b[:], start=True, stop=True)
        mrs_ps = bcps.tile([P, T], F32, tag="bc")
        nc.tensor.matmul(mrs_ps[:], ones_row[:], mrs_b[:], start=True, stop=True)
        out = []
        for t in range(CT):
            tmp = hpool.tile([P, T], F32, tag="lntmp", bufs=2, name=f"{nm}tm{t}")
            nc.vector.tensor_mul(tmp[:], src_tiles[t][:], rstd_ps[:])
            if big_out is not None:
                nc.vector.tensor_sub(big_out[:, t, :], tmp[:], mrs_ps[:])
            else:
                ht = hpool.tile([P, T], BF16, tag="lnout", bufs=CT,
                                name=f"{nm}o{t}")
                nc.vector.tensor_sub(ht[:], tmp[:], mrs_ps[:])
                out.append(ht)
        return out

    # ======================= phase A: LN1, QKV, V-local ======================
    pA = ExitStack()
    stpsA = pool("stpsA", 1, space="PSUM", stack=pA)
    bcpsA = pool("bcpsA", 2, space="PSUM", stack=pA)
    mmpsA = pool("mmpsA", 2, space="PSUM", stack=pA)
    vps = pool("vps", 1, space="PSUM", stack=pA)
    wqpool = pool("wq", 1, stack=pA)   # fp8 DoubleRow qkv weights (phase A)
    klpool = pool("kloc", 1, stack=pA)  # local K^T fp8 (gathered, then dead)

    # ---- fp8 DoubleRow qkv weights: [128, NB, 2, 3*DIM] ----
    # column-split into 3 chunks per block so the loads spread over queues
    wq_sb = wqpool.tile([P, NB, 2, 3 * DIM], F8, name="wq")
    for b in range(NB):
        src = wqkv[:, b * 2 * 3 * DIM:(b + 1) * 2 * 3 * DIM].rearrange(
            "p (i m) -> p i m", i=2)
        for ch in range(3):
            nc.sync.dma_start(
                wq_sb[:, b, :, ch * DIM:(ch + 1) * DIM],
                src[:, :, ch * DIM:(ch + 1) * DIM])

    # broadcast bv across partitions (once)
    bv_b = const.tile([1, DIM], BF16)
    nc.vector.tensor_copy(bv_b[:], bv_sb[:])
    bv_ps = vps.tile([P, DIM], F32, tag="vps")
    nc.tensor.matmul(bv_ps[:, 0:512], ones_row[:], bv_b[:, 0:512],
                     start=True, stop=True)
    nc.tensor.matmul(bv_ps[:, 512:DIM], ones_row[:], bv_b[:, 512:DIM],
                     start=True, stop=True)
    nc.vector.tensor_copy(bv_bc[:], bv_ps[:])

    h1 = hpool.tile([P, CT, T], F8, name="h1big")
    layernorm(x_sb, "h1", stpsA, bcpsA, big_out=h1)

    qk_sb = [qkpool.tile([P, T], F8, name=f"qk{m}") if m < CT else
             klpool.tile([P, T], F8, name=f"qk{m}") for m in range(2 * CT)]

    def qk_proj(m):
        ps = mmpsA.tile([P, T], F32, tag="mm")
        for b in range(NB):
            nc.tensor.matmul(ps[:], wq_sb[:, b, :, m * P:(m + 1) * P],
                             h1[:, 2 * b:2 * b + 2, :],
                             start=(b == 0), stop=(b == NB - 1), perf_mode=DR)
        nc.vector.tensor_scalar_add(qk_sb[m][:], ps[:], bqk_sb[:, m:m + 1])

    KSZ = P * T                      # 65536 elems: this pair's K^T shard
    VSZ = T * 2 * DH                 # 65536 elems: this pair's V shard
    PRSZ = KSZ + VSZ
    kv_out = []

    def gather(pr):
        kv_in_pr = dram.tile([PRSZ], F8, name=f"kvi{pr}")
        kv_out_pr = dram.tile([N_CORES * PRSZ], F8, name=f"kvo{pr}",
                              addr_space="Shared")
        nc.sync.dma_start(kv_in_pr[0:KSZ], qk_sb[CT + pr][:])
        for mt in range(LPC):
            nc.sync.dma_start(
                kv_in_pr[KSZ + mt * P * 2 * DH:KSZ + (mt + 1) * P * 2 * DH],
                v_sb[mt][:, 2 * pr * DH:(2 * pr + 2) * DH])
        nc.gpsimd.collective_compute(
            "AllGather", ALU.bypass,
            replica_groups=[list(range(N_CORES))],
            ins=[kv_in_pr[:]], outs=[kv_out_pr[:]])
        kv_out.append(kv_out_pr)

    v_sb = [vpool.tile([P, DIM], F8, name=f"v{mt}") for mt in range(LPC)]

    def v_proj(n0, nw):
        for mt in range(LPC):
            ps = vps.tile([P, 512], F32, tag="vps")
            for b in range(NB):
                nc.tensor.matmul(
                    ps[:, 0:nw],
                    h1[:, 2 * b:2 * b + 2, mt * P:(mt + 1) * P],
                    wq_sb[:, b, :, 2 * DIM + n0:2 * DIM + n0 + nw],
                    start=(b == 0), stop=(b == NB - 1), perf_mode=DR)
            nc.vector.scalar_tensor_tensor(v_sb[mt][:, n0:n0 + nw], ps[:, 0:nw],
                                           1.0, bv_bc[:, n0:n0 + nw],
                                           ALU.mult, ALU.add)

    # pair 0's K tile and V columns first so its gather launches ~30us
    # earlier; the rest of K/V (and gathers 1-5) follow, then Q
    qk_proj(CT)
    v_proj(0, P)
    gather(0)
    for m in range(CT + 1, 2 * CT):
        qk_proj(m)
    v_proj(P, 384)
    v_proj(512, 256)
    for pr in range(1, PAIRS):
        gather(pr)

    # Q projections run while the gathers are in flight
    for m in range(CT):
        qk_proj(m)

    pA.close()

    # proj weights: DMA emitted at pair 1 (below) so pair 0's gathered K/V
    # loads aren't queued behind it
    wp_sb = wppool.tile([P, NB, 2, DIM], F8, name="wp")

    def load_wproj():
        for b in range(NB):
            nc.sync.dma_start(
                wp_sb[:, b, :, :],
                wproj[:, b * 2 * DIM:(b + 1) * 2 * DIM].rearrange(
                    "p (i m) -> p i m", i=2))

    def load_kpair(pr):
        kt_ = kpair.tile([P, N_TOK], F8, tag="kp", name=f"kp{pr}")
        for c in range(N_CORES):
            src = kv_out[pr][c * PRSZ:c * PRSZ + KSZ]
            nc.sync.dma_start(kt_[:, c * T:(c + 1) * T],
                              src.rearrange("(p q) -> p q", q=T))
        return kt_

    def load_vpair(pr):
        # V_aug layout: [p, j(16), slot(2), VSTRIDE] where the VSTRIDE group
        # holds head A cols 0:64, A-ones at 64, head B cols 80:144, B-ones
        # at 144.  Key of (p, j, slot) = 128*(2j + slot) + p.
        vt = vpair.tile([P, KT // 2, 2, VSTRIDE], F8, tag="vp", name=f"vp{pr}")
        for c in range(N_CORES):
            src = kv_out[pr][c * PRSZ + KSZ:c * PRSZ + KSZ + VSZ]
            src4 = src.rearrange("(jl i p f) -> p jl i f", i=2, p=P, f=2 * DH)
            for h in range(2):
                nc.sync.dma_start(
                    vt[:, 2 * c:2 * c + 2, :, h * HOFF:h * HOFF + DH],
                    src4[:, :, :, h * DH:(h + 1) * DH])
        nc.gpsimd.memset(vt[:, :, :, DH:DH + 1], 1.0)
        nc.gpsimd.memset(vt[:, :, :, HOFF + DH:HOFF + DH + 1], 1.0)
        return vt

    # ======================= phase B: attention ==============================
    pB = ExitStack()
    scps = pool("scps", 3, space="PSUM", stack=pB)
    accps = pool("accps", 2, space="PSUM", stack=pB)

    JT = KT // 2
    ao_big = aopool.tile([P, CT, T], F8, name="aobig")
    pending = None  # previous pair's deferred normalization
    for pr in range(PAIRS):
        q_tile = qk_sb[pr]
        k_tile = load_kpair(pr)
        v_tile = load_vpair(pr)
        if pr == 1:
            load_wproj()
        # previous pair's normalization BEFORE reusing its acc psum slots
        # (accps bufs=2: acc(pr) recycles acc(pr-1)'s banks)
        if pending is not None:
            pending()
        acc_a = accps.tile([P, T], F32, tag="acc", name=f"acca{pr}")
        acc_b = accps.tile([P, T], F32, tag="acc", name=f"accb{pr}")

        def scores(kt, pr=pr, k_tile=k_tile, q_tile=q_tile):
            sc = scps.tile([P, 2 * T], F32, tag="sc", name=f"sc{pr}_{kt}")
            nc.tensor.matmul(sc[:, 0:T], k_tile[0:DH, kt * P:(kt + 1) * P],
                             q_tile[0:DH, :], start=True, stop=True)
            nc.tensor.matmul(sc[:, T:2 * T], k_tile[DH:P, kt * P:(kt + 1) * P],
                             q_tile[DH:P, :], start=True, stop=True)
            return sc

        def av(j, pt, acc_a=acc_a, acc_b=acc_b, v_tile=v_tile):
            nc.tensor.matmul(acc_a[0:DH + 1, :], v_tile[:, j, :, 0:DH + 1],
                             pt[:, :, 0:T], start=(j == 0), stop=(j == JT - 1),
                             perf_mode=DR)
            nc.tensor.matmul(acc_b[0:DH + 1, :],
                             v_tile[:, j, :, HOFF:HOFF + DH + 1],
                             pt[:, :, T:2 * T], start=(j == 0),
                             stop=(j == JT - 1), perf_mode=DR)

        prev_pt = None
        for j in range(JT):
            sc0 = scores(2 * j)
            pt = ptpool.tile([P, 2, 2 * T], F8, tag="pt", name=f"pt{pr}_{j}")
            nc.scalar.activation(pt[:, 0, :], sc0[:], AF.Exp,
                                 bias=zero_bias[:], scale=ACT_SCALE)
            sc1 = scores(2 * j + 1)
            if j == JT - 1:
                # 17/15 Act/DVE split
                nc.scalar.activation(pt[:, 1, :], sc1[:], AF.Exp,
                                     bias=zero_bias[:], scale=ACT_SCALE)
            else:
                nc.vector.tensor_scalar(pt[:, 1, :].bitcast(I8), sc1[:],
                                        A_CONST, B_CONST, ALU.mult, ALU.add)
            if j > 0:
                av(j - 1, prev_pt)
            prev_pt = pt
        av(JT - 1, prev_pt)

        def mk_finish(pr, acc_a, acc_b):
            def fin():
                # softmax denominators: Act stages sumexp to SBUF, GpSimd
                # computes 1/x (pow -1) + the partition broadcast; the busy
                # Vector engine only does the final normalize multiplies
                for half, acc in ((0, acc_a), (1, acc_b)):
                    rec_b = stat.tile([1, T], BF16, tag="recb", bufs=2,
                                      name=f"rb{pr}{half}")
                    with nc.allow_low_precision(
                            "softmax denominator broadcast is bf16 anyway"):
                        nc.vector.reciprocal(rec_b[:], acc[DH:DH + 1, :])
                    bc_sb = stat.tile([DH, T], BF16, tag="bcsb", bufs=2,
                                      name=f"bs{pr}{half}")
                    nc.gpsimd.partition_broadcast(bc_sb[:], rec_b[:])
                    nc.vector.tensor_mul(
                        ao_big[half * DH:(half + 1) * DH, pr, :],
                        acc[0:DH, :], bc_sb[:])
            return fin

        pending = mk_finish(pr, acc_a, acc_b)

    pending()
    pB.close()

    # ======================= phase C1: proj + residual + LN2 =================
    pC1 = ExitStack()
    stpsC = pool("stpsC", 1, space="PSUM", stack=pC1)
    bcpsC = pool("bcpsC", 2, space="PSUM", stack=pC1)
    mmpsC = pool("mmpsC", 2, space="PSUM", stack=pC1)

    x1_sb = [x1pool.tile([P, T], F32, name=f"x1_{m}") for m in range(CT)]
    s2ps = stpsC.tile([1, T], F32, tag="s")
    sq2ps = stpsC.tile([1, T], F32, tag="sq")
    for m in range(CT):
        ps = mmpsC.tile([P, T], F32, tag="mm")
        for b in range(NB):
            nc.tensor.matmul(ps[:], wp_sb[:, b, :, m * P:(m + 1) * P],
                             ao_big[:, 2 * b:2 * b + 2, :],
                             start=(b == 0), stop=(b == NB - 1), perf_mode=DR)
        tmp = hpool.tile([P, T], F32, tag="lntmp", bufs=2, name=f"prt{m}")
        nc.vector.scalar_tensor_tensor(tmp[:], ps[:], DS, x_sb[m][:],
                                       ALU.mult, ALU.add)
        nc.vector.tensor_scalar_add(x1_sb[m][:], tmp[:], bproj_sb[:, m:m + 1])
        # LN2 stats interleave with the proj loop (x1 tile m just completed)
        ln_stats(x1_sb[m], m, "h2", s2ps, sq2ps)

    h2 = ln_finish(x1_sb, "h2", s2ps, sq2ps, bcpsC)
    pC1.close()

    # ======================= phase C2: MLP ===================================
    pC2 = ExitStack()
    mmpsM = pool("mmpsM", 2, space="PSUM", stack=pC2)
    fc2ps = pool("fc2ps", CT, space="PSUM", stack=pC2)

    fc1_bands = [wband(kt, wfc1, HIDDEN) for kt in range(CT)]
    g_sb = [gpool.tile([P, T], BF16, tag="g", name=f"g{m}")
            for m in range(HIDDEN // P)]
    o_ps = [fc2ps.tile([P, T], F32, tag="oacc", name=f"ops{m}") for m in range(CT)]

    def fc2_step(kt):
        band = wband(kt, wfc2, DIM)
        for m in range(CT):
            nc.tensor.matmul(o_ps[m][:], band[:, m * P:(m + 1) * P], g_sb[kt][:],
                             start=(kt == 0), stop=(kt == HIDDEN // P - 1))

    # fc2's kt-accumulation interleaves with fc1's m-loop: fc2(kt) only
    # needs g_sb[kt], so the PE stream never waits on a gelu
    for m in range(HIDDEN // P):
        ps = mmpsM.tile([P, T], F32, tag="mm")
        for kt in range(CT):
            nc.tensor.matmul(ps[:], fc1_bands[kt][:, m * P:(m + 1) * P],
                             h2[kt][:], start=(kt == 0), stop=(kt == CT - 1))
        nc.scalar.activation(g_sb[m][:], ps[:], GELU_FUNC,
                             bias=bfc1_sb[:, m:m + 1], scale=1.0)
        if m >= 2:
            fc2_step(m - 2)
    fc2_step(HIDDEN // P - 2)
    fc2_step(HIDDEN // P - 1)
    for m in range(CT):
        ot = opool.tile([P, T], F32, tag="ot", name=f"ot{m}")
        nc.vector.scalar_tensor_tensor(ot[:], o_ps[m][:], bfc2_sb[:, m:m + 1],
                                       x1_sb[m][:], ALU.add, ALU.add)
        nc.sync.dma_start(outt[m * P:(m + 1) * P, :], ot[:])
    pC2.close()
    top.close()


def _interleave_dr(w):
    """[DIM_in, M] -> DoubleRow stationary layout [128, NB*2*M] where
    weight row (p, b, i) holds input feature 256*b + 128*i + p."""
    din, m = w.shape
    nb = din // 256
    w4 = w.reshape(nb, 2, P, m)          # [b, i, p, m]
    w4 = w4.transpose(2, 0, 1, 3)        # [p, b, i, m]
    return np.ascontiguousarray(w4.reshape(P, nb * 2 * m))


def _prepare_in_maps(inputs):
    x = np.asarray(inputs["x"], np.float32)
    g1 = np.asarray(inputs["g1"], np.float32)
    b1 = np.asarray(inputs["b1"], np.float32)
    g2 = np.asarray(inputs["g2"], np.float32)
    b2 = np.asarray(inputs["b2"], np.float32)
    w_qkv = np.asarray(inputs["w_qkv"], np.float32)
    w_proj = np.asarray(inputs["w_proj"], np.float32)
    b_proj = np.asarray(inputs["b_proj"], np.float32)
    w_fc1 = np.asarray(inputs["w_fc1"], np.float32)
    b_fc1 = np.asarray(inputs["b_fc1"], np.float32)
    w_fc2 = np.asarray(inputs["w_fc2"], np.float32)
    b_fc2 = np.asarray(inputs["b_fc2"], np.float32)

    bf = ml_dtypes.bfloat16
    f8 = mybir.dt.np(F8)
    wqkv_eff = _interleave_dr(g1[:, None] * w_qkv * WS).astype(f8)
    bqkv_eff = (b1 @ w_qkv).astype(np.float32) * WS
    wproj_eff = _interleave_dr(w_proj * WS).astype(f8)
    wfc1_eff = (g2[:, None] * w_fc1).astype(bf)
    bfc1_eff = (b_fc1 + b2 @ w_fc1).astype(np.float32)

    shared = {
        "wqkv": wqkv_eff,
        "bqk": np.ascontiguousarray(bqkv_eff[:2 * DIM]),
        "bv": np.ascontiguousarray(bqkv_eff[2 * DIM:]),
        "wproj": wproj_eff,
        "bproj": b_proj,
        "wfc1": wfc1_eff,
        "bfc1": bfc1_eff,
        "wfc2": w_fc2.astype(bf),
        "bfc2": b_fc2,
    }
    in_maps = []
    for c in range(N_CORES):
        xs = np.ascontiguousarray(x[0, c * T:(c + 1) * T, :].T)
        in_maps.append({"xt": xs, **shared})
    return in_maps


def _install_ntff_hook():
    """The agent image's antenv lacks axon_hooks; synthesize it so
    BASS_TRACE=1 profiling works (and its absence never crashes)."""
    import types
    try:
        from antenv.axon_hooks import get_axon_ntff_profile_hook  # noqa: F401
        return
    except ImportError:
        pass
    try:
        import antenv
        mod = types.ModuleType("antenv.axon_hooks")
        _h = [None]
        mod.set_axon_ntff_profile_hook = lambda h: _h.__setitem__(0, h)
        mod.get_axon_ntff_profile_hook = lambda: _h[0]
        sys.modules["antenv.axon_hooks"] = mod
        antenv.axon_hooks = mod
        try:
            from trn_agent_boot.trn_boot import _ntff_profile_via_ctypes
            so = "/opt/axon/libaxon_pjrt.so"
            if os.path.exists(so):
                mod.set_axon_ntff_profile_hook(_ntff_profile_via_ctypes(so))
        except Exception:
            pass
    except Exception:
        pass


def kernel(**inputs):
    global _CACHED_NC, LAST_RESULTS
    from concourse.bass_utils import run_bass_kernel_spmd

    _install_ntff_hook()

    if _CACHED_NC is None:
        _CACHED_NC = build_nc()
    nc = _CACHED_NC
    in_maps = _prepare_in_maps(inputs)
    res = run_bass_kernel_spmd(nc, in_maps, list(range(N_CORES)))
    LAST_RESULTS = res
    out = np.empty((1, N_TOK, DIM), np.float32)
    for c in range(N_CORES):
        out[0, c * T:(c + 1) * T, :] = res.results[c]["outt"].T
    return out


# revision 70
# speedup vs baseline: 1.1504x; 1.0082x over previous
"""Trainium2 Bass kernel: pre-norm transformer block (dense_transformer).

Reference (per token row x of [4096, 768]):
  h1 = LN(x; g1, b1);  qkv = h1 @ w_qkv;  attention (12 heads, dh=64, softmax)
  x1 = x + attn_out @ w_proj + b_proj
  h2 = LN(x1; g2, b2); out = x1 + gelu(h2 @ w_fc1 + b_fc1) @ w_fc2 + b_fc2

Sharding: sequence (data) parallel - each of 8 cores owns 512 tokens.  K/V of
the full sequence are exchanged with one AllGather collective per head pair;
everything else is core-local (no all-reduces at all).

Perf design on top of the bf16 baseline:
  * The softmax exp (25.2M elements/core) is the attention wall on the Act
    engine; it is now split between the Act engine (true Exp) and the Vector
    engine, which computes exp with a Schraudolph bit-trick: byte =
    score*(8*log2e*scale) + 56.5 converted to int8 IS the fp8-e4m3 encoding
    of exp(score*scale).  The softmax denominator (ones column in V_aug)
    normalizes the same quantized weights, so the error largely cancels.
  * Attention runs in fp8-e4m3: Q/K/V/P tiles are fp8.  AV uses DoubleRow
    fp8 matmuls (2 key-tiles contracted per pass, ~1.4x the bf16 rate);
    scores use PE row-tiling (two dh=64 heads in the two PE row halves).
  * QKV and proj matmuls use DoubleRow fp8 with host-interleaved weights
    ([128, nb, 2, M] layout; w' = diag(g)*w*32 in e4m3, the 32x pre-scale
    keeps the 0.02-scale weights out of the e4m3 subnormal range; the 1/1024
    descale folds into the exp scale and the proj residual step).
  * MLP (fc1/fc2) stays bf16: fp8 there costs ~1.5e-2 absmax error.
  * K/V are exchanged in fp8: half the collective bytes of bf16.

Matmul operands are bf16/fp8 (fp32 matmuls are 4x slower on the PE);
statistics, softmax sums, and the residual stream stay fp32.
"""

import os
import sys

import numpy as np

for _p in ("/opt/trn_rl_repo",):
    if os.path.isdir(_p) and _p not in sys.path:
        sys.path.insert(0, _p)

os.environ.setdefault("MYCRO_LOCAL_CACHE", "1")

import ml_dtypes  # noqa: E402

import concourse.bass as bass  # noqa: E402
import concourse.mybir as mybir  # noqa: E402
import concourse.tile as tile  # noqa: E402
from concourse import bacc  # noqa: E402

DIM = 768
N_TOK = 4096
HEADS = 12
DH = 64
HIDDEN = 4 * DIM
EPS = 1e-5
N_CORES = 8
T = N_TOK // N_CORES          # 512 local tokens per core
P = 128
CT = DIM // P                 # 6 feature tiles
KT = N_TOK // P               # 32 key tiles
LPC = T // P                  # 4 local token tiles
SCALE = DH ** -0.5
PAIRS = HEADS // 2
NB = CT // 2                  # 3 DoubleRow feature blocks of 256

WS = 32.0                     # fp8 weight pre-scale for wqkv / wproj
DS = 1.0 / (WS * WS)          # descale after a fp8xfp8 (x32 * x32) matmul
ACT_SCALE = SCALE * DS        # exp() argument scale on the Act engine
LOG2E = 1.4426950408889634
A_CONST = 8.0 * LOG2E * ACT_SCALE   # Schraudolph multiplier (e4m3 grid)
B_CONST = 56.5                      # e4m3 exponent-bias offset (+0.5 trunc)

VSTRIDE = 160                 # V_aug free stride per (j, slot); %16 == 0
HOFF = 80                     # head B column offset inside a V_aug group

F32 = mybir.dt.float32
BF16 = mybir.dt.bfloat16
F8 = mybir.dt.float8e4
I8 = mybir.dt.int8
AF = mybir.ActivationFunctionType
ALU = mybir.AluOpType
DR = mybir.MatmulPerfMode.DoubleRow

_CACHED_NC = None
LAST_RESULTS = None
GELU_FUNC = AF.Gelu  # sim_test swaps this (CoreSim lacks Gelu)


def build_nc():
    nc = bacc.Bacc(num_devices=N_CORES)

    xt = nc.declare_dram_parameter("xt", [DIM, T], F32, isOutput=False)
    wqkv = nc.declare_dram_parameter("wqkv", [P, 2 * NB * 3 * DIM], F8,
                                     isOutput=False)
    bqk = nc.declare_dram_parameter("bqk", [2 * DIM], F32, isOutput=False)
    bv = nc.declare_dram_parameter("bv", [DIM], F32, isOutput=False)
    wproj = nc.declare_dram_parameter("wproj", [P, 2 * NB * DIM], F8,
                                      isOutput=False)
    bproj = nc.declare_dram_parameter("bproj", [DIM], F32, isOutput=False)
    wfc1 = nc.declare_dram_parameter("wfc1", [DIM, HIDDEN], BF16, isOutput=False)
    bfc1 = nc.declare_dram_parameter("bfc1", [HIDDEN], F32, isOutput=False)
    wfc2 = nc.declare_dram_parameter("wfc2", [HIDDEN, DIM], BF16, isOutput=False)
    bfc2 = nc.declare_dram_parameter("bfc2", [DIM], F32, isOutput=False)
    outt = nc.declare_dram_parameter("outt", [DIM, T], F32, isOutput=True)

    with tile.TileContext(nc) as tc:
        _emit(nc, tc, xt, wqkv, bqk, bv, wproj, bproj, wfc1, bfc1, wfc2, bfc2,
              outt)
    # Bacc defers register allocation + event-semaphore splitting to
    # compile(); the PJRT exec path serializes the module as-is, so run
    # them now.
    nc.finalize()
    return nc


def _emit(nc, tc, xt, wqkv, bqk, bv, wproj, bproj, wfc1, bfc1, wfc2, bfc2, outt):
    from contextlib import ExitStack

    top = ExitStack()

    def pool(name, bufs, space="SBUF", stack=None):
        return (stack or top).enter_context(
            tc.tile_pool(name=name, bufs=bufs, space=space))

    # ---- long-lived SBUF pools ----
    const = pool("const", 1)
    xpool = pool("x", 1)               # x^T fp32, lives to the proj residual
    hpool = pool("h", 1)               # LN temporaries + normalized output
    qkpool = pool("qk", 1)             # Q^T fp8 (lives through phase B)
    vpool = pool("vloc", 1)            # local V token-major fp8
    kpair = pool("kpair", 2)           # streamed gathered K^T [128, 4096] fp8
    vpair = pool("vpair", 2)           # streamed gathered V_aug fp8
    aopool = pool("ao", 1)             # attention out^T fp8 [128, CT, T]
    x1pool = pool("x1", 1)             # post-attention residual fp32
    gpool = pool("g", 12)              # gelu activations bf16
    opool = pool("o", 3)               # output fp32 staging
    wpool = pool("w", 7)               # bf16 weight bands, one shared tag
    wppool = pool("wp", 1)             # fp8 DoubleRow proj weights
    stat = pool("stat", 1)             # small [1, T] statistics
    ptpool = pool("pt", 4)             # P = exp(scores) fp8 [128, 2, 1024]
    dram = pool("dram", 1, space="DRAM")

    # ---- constants / bias vectors ----
    ones_stat = const.tile([P, 1], BF16)
    nc.vector.memset(ones_stat[:], 1.0)
    ones_row = const.tile([1, P], BF16)
    nc.vector.memset(ones_row[:], 1.0)
    zero_bias = const.tile([P, 1], F32)
    nc.vector.memset(zero_bias[:], 0.0)
    eps_tile = const.tile([1, 1], F32)
    nc.vector.memset(eps_tile[:], EPS)

    bqk_sb = const.tile([P, 2 * DIM // P], F32)
    nc.sync.dma_start(bqk_sb[:], bqk.rearrange("(t p) -> p t", p=P))
    bv_sb = const.tile([1, DIM], F32)
    nc.sync.dma_start(bv_sb[:], bv[None, :])
    bproj_sb = const.tile([P, CT], F32)
    nc.sync.dma_start(bproj_sb[:], bproj.rearrange("(t p) -> p t", p=P))
    bfc1_sb = const.tile([P, HIDDEN // P], F32)
    nc.sync.dma_start(bfc1_sb[:], bfc1.rearrange("(t p) -> p t", p=P))
    bfc2_sb = const.tile([P, CT], F32)
    nc.sync.dma_start(bfc2_sb[:], bfc2.rearrange("(t p) -> p t", p=P))
    bv_bc = const.tile([P, DIM], F32)

    # ---- x^T ----  (column-split DMAs: one queue per chunk halves latency)
    x_sb = [xpool.tile([P, T], F32, name=f"x{t}") for t in range(CT)]
    for t in range(CT):
        for half in range(2):
            nc.sync.dma_start(
                x_sb[t][:, half * (T // 2):(half + 1) * (T // 2)],
                xt[t * P:(t + 1) * P, half * (T // 2):(half + 1) * (T // 2)])

    def wband(kt, src, width):
        b = wpool.tile([P, HIDDEN], BF16, tag="wband", name=f"wb{kt}")
        bb = b[:, :width]
        nc.sync.dma_start(bb, src[kt * P:(kt + 1) * P, :])
        return bb

    # ---- layernorm: (x - mean) * rsqrt(var + eps); out fp8 big tile or
    # ---- per-tile bf16 list.  Split so callers can interleave the stats
    # ---- pass with the producer of src_tiles. ----
    def ln_stats(src_tile, t, nm, s_ps, sq_ps):
        xb = hpool.tile([P, T], BF16, tag="lnxb", bufs=2, name=f"{nm}xb{t}")
        nc.vector.tensor_copy(xb[:], src_tile[:])
        xsq = hpool.tile([P, T], BF16, tag="lnxsq", bufs=2, name=f"{nm}sq{t}")
        nc.vector.tensor_mul(xsq[:], xb[:], xb[:])
        nc.tensor.matmul(s_ps[:], ones_stat[:], xb[:],
                         start=(t == 0), stop=(t == CT - 1))
        nc.tensor.matmul(sq_ps[:], ones_stat[:], xsq[:],
                         start=(t == 0), stop=(t == CT - 1))

    def layernorm(src_tiles, nm, stps, bcps, big_out=None):
        s_ps = stps.tile([1, T], F32, tag="s")
        sq_ps = stps.tile([1, T], F32, tag="sq")
        for t in range(CT):
            ln_stats(src_tiles[t], t, nm, s_ps, sq_ps)
        return ln_finish(src_tiles, nm, s_ps, sq_ps, bcps, big_out)

    def ln_finish(src_tiles, nm, s_ps, sq_ps, bcps, big_out=None):
        ssum = stat.tile([1, T], F32, tag="lnf", bufs=5, name=f"{nm}sum")
        nc.vector.tensor_copy(ssum[:], s_ps[:])
        t1 = stat.tile([1, T], F32, tag="lnf", bufs=5, name=f"{nm}t1")
        nc.vector.scalar_tensor_tensor(t1[:], ssum[:], 1.0 / DIM, ssum[:],
                                       ALU.mult, ALU.mult)
        t2 = stat.tile([1, T], F32, tag="lnf", bufs=5, name=f"{nm}t2")
        nc.vector.tensor_sub(t2[:], sq_ps[:], t1[:])
        sdev = stat.tile([1, T], F32, tag="lnf", bufs=5, name=f"{nm}sdev")
        nc.scalar.activation(sdev[:], t2[:], AF.Sqrt,
                             bias=eps_tile[:], scale=1.0 / DIM)
        rstd = stat.tile([1, T], F32, tag="lnf", bufs=5, name=f"{nm}rstd")
        nc.vector.reciprocal(rstd[:], sdev[:])
        rstd_b = stat.tile([1, T], BF16, tag="lnb", bufs=2, name=f"{nm}rstdb")
        nc.vector.tensor_copy(rstd_b[:], rstd[:])
        mrs_b = stat.tile([1, T], BF16, tag="lnb", bufs=2, name=f"{nm}mrsb")
        nc.vector.scalar_tensor_tensor(mrs_b[:], ssum[:], 1.0 / DIM, rstd[:],
                                       ALU.mult, ALU.mult)
        rstd_ps = bcps.tile([P, T], F32, tag="bc")
        nc.tensor.matmul(rstd_ps[:], ones_row[:], rstd_b[:], start=True, stop=True)
        mrs_ps = bcps.tile([P, T], F32, tag="bc")
        nc.tensor.matmul(mrs_ps[:], ones_row[:], mrs_b[:], start=True, stop=True)
        out = []
        for t in range(CT):
            tmp = hpool.tile([P, T], F32, tag="lntmp", bufs=2, name=f"{nm}tm{t}")
            nc.vector.tensor_mul(tmp[:], src_tiles[t][:], rstd_ps[:])
            if big_out is not None:
                nc.vector.tensor_sub(big_out[:, t, :], tmp[:], mrs_ps[:])
            else:
                ht = hpool.tile([P, T], BF16, tag="lnout", bufs=CT,
                                name=f"{nm}o{t}")
                nc.vector.tensor_sub(ht[:], tmp[:], mrs_ps[:])
                out.append(ht)
        return out

    # ======================= phase A: LN1, QKV, V-local ======================
    pA = ExitStack()
    stpsA = pool("stpsA", 1, space="PSUM", stack=pA)
    bcpsA = pool("bcpsA", 2, space="PSUM", stack=pA)
    mmpsA = pool("mmpsA", 2, space="PSUM", stack=pA)
    vps = pool("vps", 1, space="PSUM", stack=pA)
    wqpool = pool("wq", 1, stack=pA)   # fp8 DoubleRow qkv weights (phase A)
    klpool = pool("kloc", 1, stack=pA)  # local K^T fp8 (gathered, then dead)

    # ---- fp8 DoubleRow qkv weights: [128, NB, 2, 3*DIM] ----
    # column-split into 3 chunks per block so the loads spread over queues
    wq_sb = wqpool.tile([P, NB, 2, 3 * DIM], F8, name="wq")
    for b in range(NB):
        src = wqkv[:, b * 2 * 3 * DIM:(b + 1) * 2 * 3 * DIM].rearrange(
            "p (i m) -> p i m", i=2)
        for ch in range(3):
            nc.sync.dma_start(
                wq_sb[:, b, :, ch * DIM:(ch + 1) * DIM],
                src[:, :, ch * DIM:(ch + 1) * DIM])

    # broadcast bv across partitions (once)
    bv_b = const.tile([1, DIM], BF16)
    nc.vector.tensor_copy(bv_b[:], bv_sb[:])
    bv_ps = vps.tile([P, DIM], F32, tag="vps")
    nc.tensor.matmul(bv_ps[:, 0:512], ones_row[:], bv_b[:, 0:512],
                     start=True, stop=True)
    nc.tensor.matmul(bv_ps[:, 512:DIM], ones_row[:], bv_b[:, 512:DIM],
                     start=True, stop=True)
    nc.vector.tensor_copy(bv_bc[:], bv_ps[:])

    h1 = hpool.tile([P, CT, T], F8, name="h1big")
    layernorm(x_sb, "h1", stpsA, bcpsA, big_out=h1)

    qk_sb = [qkpool.tile([P, T], F8, name=f"qk{m}") if m < CT else
             klpool.tile([P, T], F8, name=f"qk{m}") for m in range(2 * CT)]

    def qk_proj(m):
        ps = mmpsA.tile([P, T], F32, tag="mm")
        for b in range(NB):
            nc.tensor.matmul(ps[:], wq_sb[:, b, :, m * P:(m + 1) * P],
                             h1[:, 2 * b:2 * b + 2, :],
                             start=(b == 0), stop=(b == NB - 1), perf_mode=DR)
        nc.vector.tensor_scalar_add(qk_sb[m][:], ps[:], bqk_sb[:, m:m + 1])

    KSZ = P * T                      # 65536 elems: this pair's K^T shard
    VSZ = T * 2 * DH                 # 65536 elems: this pair's V shard
    PRSZ = KSZ + VSZ
    kv_out = []

    def gather(pr):
        kv_in_pr = dram.tile([PRSZ], F8, name=f"kvi{pr}")
        kv_out_pr = dram.tile([N_CORES * PRSZ], F8, name=f"kvo{pr}",
                              addr_space="Shared")
        nc.sync.dma_start(kv_in_pr[0:KSZ], qk_sb[CT + pr][:])
        for mt in range(LPC):
            nc.sync.dma_start(
                kv_in_pr[KSZ + mt * P * 2 * DH:KSZ + (mt + 1) * P * 2 * DH],
                v_sb[mt][:, 2 * pr * DH:(2 * pr + 2) * DH])
        nc.gpsimd.collective_compute(
            "AllGather", ALU.bypass,
            replica_groups=[list(range(N_CORES))],
            ins=[kv_in_pr[:]], outs=[kv_out_pr[:]])
        kv_out.append(kv_out_pr)

    v_sb = [vpool.tile([P, DIM], F8, name=f"v{mt}") for mt in range(LPC)]

    def v_proj(n0, nw):
        for mt in range(LPC):
            ps = vps.tile([P, 512], F32, tag="vps")
            for b in range(NB):
                nc.tensor.matmul(
                    ps[:, 0:nw],
                    h1[:, 2 * b:2 * b + 2, mt * P:(mt + 1) * P],
                    wq_sb[:, b, :, 2 * DIM + n0:2 * DIM + n0 + nw],
                    start=(b == 0), stop=(b == NB - 1), perf_mode=DR)
            nc.vector.scalar_tensor_tensor(v_sb[mt][:, n0:n0 + nw], ps[:, 0:nw],
                                           1.0, bv_bc[:, n0:n0 + nw],
                                           ALU.mult, ALU.add)

    # pair 0's K tile and V columns first so its gather launches ~30us
    # earlier; the rest of K/V (and gathers 1-5) follow, then Q
    qk_proj(CT)
    v_proj(0, P)
    gather(0)
    for m in range(CT + 1, 2 * CT):
        qk_proj(m)
    v_proj(P, 384)
    v_proj(512, 256)
    for pr in range(1, PAIRS):
        gather(pr)

    # Q projections run while the gathers are in flight
    for m in range(CT):
        qk_proj(m)

    pA.close()

    # proj weights: DMA emitted at pair 1 (below) so pair 0's gathered K/V
    # loads aren't queued behind it
    wp_sb = wppool.tile([P, NB, 2, DIM], F8, name="wp")

    def load_wproj():
        for b in range(NB):
            nc.sync.dma_start(
                wp_sb[:, b, :, :],
                wproj[:, b * 2 * DIM:(b + 1) * 2 * DIM].rearrange(
                    "p (i m) -> p i m", i=2))

    def load_kpair(pr):
        kt_ = kpair.tile([P, N_TOK], F8, tag="kp", name=f"kp{pr}")
        for c in range(N_CORES):
            src = kv_out[pr][c * PRSZ:c * PRSZ + KSZ]
            nc.sync.dma_start(kt_[:, c * T:(c + 1) * T],
                              src.rearrange("(p q) -> p q", q=T))
        return kt_

    def load_vpair(pr):
        # V_aug layout: [p, j(16), slot(2), VSTRIDE] where the VSTRIDE group
        # holds head A cols 0:64, A-ones at 64, head B cols 80:144, B-ones
        # at 144.  Key of (p, j, slot) = 128*(2j + slot) + p.
        vt = vpair.tile([P, KT // 2, 2, VSTRIDE], F8, tag="vp", name=f"vp{pr}")
        for c in range(N_CORES):
            src = kv_out[pr][c * PRSZ + KSZ:c * PRSZ + KSZ + VSZ]
            src4 = src.rearrange("(jl i p f) -> p jl i f", i=2, p=P, f=2 * DH)
            for h in range(2):
                nc.sync.dma_start(
                    vt[:, 2 * c:2 * c + 2, :, h * HOFF:h * HOFF + DH],
                    src4[:, :, :, h * DH:(h + 1) * DH])
        nc.gpsimd.memset(vt[:, :, :, DH:DH + 1], 1.0)
        nc.gpsimd.memset(vt[:, :, :, HOFF + DH:HOFF + DH + 1], 1.0)
        return vt

    # ======================= phase B: attention ==============================
    pB = ExitStack()
    scps = pool("scps", 3, space="PSUM", stack=pB)
    accps = pool("accps", 2, space="PSUM", stack=pB)

    JT = KT // 2
    ao_big = aopool.tile([P, CT, T], F8, name="aobig")
    pending = None  # previous pair's deferred normalization
    for pr in range(PAIRS):
        q_tile = qk_sb[pr]
        k_tile = load_kpair(pr)
        v_tile = load_vpair(pr)
        if pr == 1:
            load_wproj()
        acc_a = accps.tile([P, T], F32, tag="acc", name=f"acca{pr}")
        acc_b = accps.tile([P, T], F32, tag="acc", name=f"accb{pr}")

        def scores(kt, pr=pr, k_tile=k_tile, q_tile=q_tile):
            sc = scps.tile([P, 2 * T], F32, tag="sc", name=f"sc{pr}_{kt}")
            nc.tensor.matmul(sc[:, 0:T], k_tile[0:DH, kt * P:(kt + 1) * P],
                             q_tile[0:DH, :], start=True, stop=True)
            nc.tensor.matmul(sc[:, T:2 * T], k_tile[DH:P, kt * P:(kt + 1) * P],
                             q_tile[DH:P, :], start=True, stop=True)
            return sc

        def av(j, pt, acc_a=acc_a, acc_b=acc_b, v_tile=v_tile):
            nc.tensor.matmul(acc_a[0:DH + 1, :], v_tile[:, j, :, 0:DH + 1],
                             pt[:, :, 0:T], start=(j == 0), stop=(j == JT - 1),
                             perf_mode=DR)
            nc.tensor.matmul(acc_b[0:DH + 1, :],
                             v_tile[:, j, :, HOFF:HOFF + DH + 1],
                             pt[:, :, T:2 * T], start=(j == 0),
                             stop=(j == JT - 1), perf_mode=DR)

        prev_pt = None
        for j in range(JT):
            sc0 = scores(2 * j)
            pt = ptpool.tile([P, 2, 2 * T], F8, tag="pt", name=f"pt{pr}_{j}")
            nc.scalar.activation(pt[:, 0, :], sc0[:], AF.Exp,
                                 bias=zero_bias[:], scale=ACT_SCALE)
            sc1 = scores(2 * j + 1)
            if j == JT - 1:
                # 17/15 Act/DVE split
                nc.scalar.activation(pt[:, 1, :], sc1[:], AF.Exp,
                                     bias=zero_bias[:], scale=ACT_SCALE)
            else:
                nc.vector.tensor_scalar(pt[:, 1, :].bitcast(I8), sc1[:],
                                        A_CONST, B_CONST, ALU.mult, ALU.add)
            # previous pair's normalization, one half per round, AFTER this
            # round's DVE exp (so the new pair's pipeline restarts at once)
            # and BEFORE av(j-1) (av(0) recycles the prev pair's acc banks,
            # accps bufs=2 — each half's reads must be emitted first)
            if j < 2 and pending is not None:
                pending[j]()
            if j > 0:
                av(j - 1, prev_pt)
            prev_pt = pt
        av(JT - 1, prev_pt)

        def mk_finish(pr, acc_a, acc_b):
            def fin_half(half, acc):
                def go():
                    rec_b = stat.tile([1, T], BF16, tag="recb", bufs=2,
                                      name=f"rb{pr}{half}")
                    with nc.allow_low_precision(
                            "softmax denominator broadcast is bf16 anyway"):
                        nc.vector.reciprocal(rec_b[:], acc[DH:DH + 1, :])
                    bc_sb = stat.tile([DH, T], BF16, tag="bcsb", bufs=2,
                                      name=f"bs{pr}{half}")
                    nc.gpsimd.partition_broadcast(bc_sb[:], rec_b[:])
                    nc.vector.tensor_mul(
                        ao_big[half * DH:(half + 1) * DH, pr, :],
                        acc[0:DH, :], bc_sb[:])
                return go
            return [fin_half(0, acc_a), fin_half(1, acc_b)]

        pending = mk_finish(pr, acc_a, acc_b)

    pending[0]()
    pending[1]()
    pB.close()

    # ======================= phase C1: proj + residual + LN2 =================
    pC1 = ExitStack()
    stpsC = pool("stpsC", 1, space="PSUM", stack=pC1)
    bcpsC = pool("bcpsC", 2, space="PSUM", stack=pC1)
    mmpsC = pool("mmpsC", 2, space="PSUM", stack=pC1)

    x1_sb = [x1pool.tile([P, T], F32, name=f"x1_{m}") for m in range(CT)]
    s2ps = stpsC.tile([1, T], F32, tag="s")
    sq2ps = stpsC.tile([1, T], F32, tag="sq")
    for m in range(CT):
        ps = mmpsC.tile([P, T], F32, tag="mm")
        for b in range(NB):
            nc.tensor.matmul(ps[:], wp_sb[:, b, :, m * P:(m + 1) * P],
                             ao_big[:, 2 * b:2 * b + 2, :],
                             start=(b == 0), stop=(b == NB - 1), perf_mode=DR)
        tmp = hpool.tile([P, T], F32, tag="lntmp", bufs=2, name=f"prt{m}")
        nc.vector.scalar_tensor_tensor(tmp[:], ps[:], DS, x_sb[m][:],
                                       ALU.mult, ALU.add)
        nc.vector.tensor_scalar_add(x1_sb[m][:], tmp[:], bproj_sb[:, m:m + 1])
        # LN2 stats interleave with the proj loop (x1 tile m just completed)
        ln_stats(x1_sb[m], m, "h2", s2ps, sq2ps)

    h2 = ln_finish(x1_sb, "h2", s2ps, sq2ps, bcpsC)
    pC1.close()

    # ======================= phase C2: MLP ===================================
    pC2 = ExitStack()
    mmpsM = pool("mmpsM", 2, space="PSUM", stack=pC2)
    fc2ps = pool("fc2ps", CT, space="PSUM", stack=pC2)

    fc1_bands = [wband(kt, wfc1, HIDDEN) for kt in range(CT)]
    g_sb = [gpool.tile([P, T], BF16, tag="g", name=f"g{m}")
            for m in range(HIDDEN // P)]
    o_ps = [fc2ps.tile([P, T], F32, tag="oacc", name=f"ops{m}") for m in range(CT)]

    def fc2_step(kt):
        band = wband(kt, wfc2, DIM)
        for m in range(CT):
            nc.tensor.matmul(o_ps[m][:], band[:, m * P:(m + 1) * P], g_sb[kt][:],
                             start=(kt == 0), stop=(kt == HIDDEN // P - 1))

    # fc2's kt-accumulation interleaves with fc1's m-loop: fc2(kt) only
    # needs g_sb[kt], so the PE stream never waits on a gelu
    for m in range(HIDDEN // P):
        ps = mmpsM.tile([P, T], F32, tag="mm")
        for kt in range(CT):
            nc.tensor.matmul(ps[:], fc1_bands[kt][:, m * P:(m + 1) * P],
                             h2[kt][:], start=(kt == 0), stop=(kt == CT - 1))
        nc.scalar.activation(g_sb[m][:], ps[:], GELU_FUNC,
                             bias=bfc1_sb[:, m:m + 1], scale=1.0)
        if m >= 2:
            fc2_step(m - 2)
    fc2_step(HIDDEN // P - 2)
    fc2_step(HIDDEN // P - 1)
    for m in range(CT):
        ot = opool.tile([P, T], F32, tag="ot", name=f"ot{m}")
        nc.vector.scalar_tensor_tensor(ot[:], o_ps[m][:], bfc2_sb[:, m:m + 1],
                                       x1_sb[m][:], ALU.add, ALU.add)
        nc.sync.dma_start(outt[m * P:(m + 1) * P, :], ot[:])
    pC2.close()
    top.close()


def _interleave_dr(w):
    """[DIM_in, M] -> DoubleRow stationary layout [128, NB*2*M] where
    weight row (p, b, i) holds input feature 256*b + 128*i + p."""
    din, m = w.shape
    nb = din // 256
    w4 = w.reshape(nb, 2, P, m)          # [b, i, p, m]
    w4 = w4.transpose(2, 0, 1, 3)        # [p, b, i, m]
    return np.ascontiguousarray(w4.reshape(P, nb * 2 * m))


def _prepare_in_maps(inputs):
    x = np.asarray(inputs["x"], np.float32)
    g1 = np.asarray(inputs["g1"], np.float32)
    b1 = np.asarray(inputs["b1"], np.float32)
    g2 = np.asarray(inputs["g2"], np.float32)
    b2 = np.asarray(inputs["b2"], np.float32)
    w_qkv = np.asarray(inputs["w_qkv"], np.float32)
    w_proj = np.asarray(inputs["w_proj"], np.float32)
    b_proj = np.asarray(inputs["b_proj"], np.float32)
    w_fc1 = np.asarray(inputs["w_fc1"], np.float32)
    b_fc1 = np.asarray(inputs["b_fc1"], np.float32)
    w_fc2 = np.asarray(inputs["w_fc2"], np.float32)
    b_fc2 = np.asarray(inputs["b_fc2"], np.float32)

    bf = ml_dtypes.bfloat16
    f8 = mybir.dt.np(F8)
    wqkv_eff = _interleave_dr(g1[:, None] * w_qkv * WS).astype(f8)
    bqkv_eff = (b1 @ w_qkv).astype(np.float32) * WS
    wproj_eff = _interleave_dr(w_proj * WS).astype(f8)
    wfc1_eff = (g2[:, None] * w_fc1).astype(bf)
    bfc1_eff = (b_fc1 + b2 @ w_fc1).astype(np.float32)

    shared = {
        "wqkv": wqkv_eff,
        "bqk": np.ascontiguousarray(bqkv_eff[:2 * DIM]),
        "bv": np.ascontiguousarray(bqkv_eff[2 * DIM:]),
        "wproj": wproj_eff,
        "bproj": b_proj,
        "wfc1": wfc1_eff,
        "bfc1": bfc1_eff,
        "wfc2": w_fc2.astype(bf),
        "bfc2": b_fc2,
    }
    in_maps = []
    for c in range(N_CORES):
        xs = np.ascontiguousarray(x[0, c * T:(c + 1) * T, :].T)
        in_maps.append({"xt": xs, **shared})
    return in_maps


def _install_ntff_hook():
    """The agent image's antenv lacks axon_hooks; synthesize it so
    BASS_TRACE=1 profiling works (and its absence never crashes)."""
    import types
    try:
        from antenv.axon_hooks import get_axon_ntff_profile_hook  # noqa: F401
        return
    except ImportError:
        pass
    try:
        import antenv
        mod = types.ModuleType("antenv.axon_hooks")
        _h = [None]
        mod.set_axon_ntff_profile_hook = lambda h: _h.__setitem__(0, h)
        mod.get_axon_ntff_profile_hook = lambda: _h[0]
        sys.modules["antenv.axon_hooks"] = mod
        antenv.axon_hooks = mod
        try:
            from trn_agent_boot.trn_boot import _ntff_profile_via_ctypes
            so = "/opt/axon/libaxon_pjrt.so"
            if os.path.exists(so):
                mod.set_axon_ntff_profile_hook(_ntff_profile_via_ctypes(so))
        except Exception:
            pass
    except Exception:
        pass


def kernel(**inputs):
    global _CACHED_NC, LAST_RESULTS
    from concourse.bass_utils import run_bass_kernel_spmd

    _install_ntff_hook()

    if _CACHED_NC is None:
        _CACHED_NC = build_nc()
    nc = _CACHED_NC
    in_maps = _prepare_in_maps(inputs)
    res = run_bass_kernel_spmd(nc, in_maps, list(range(N_CORES)))
    LAST_RESULTS = res
    out = np.empty((1, N_TOK, DIM), np.float32)
    for c in range(N_CORES):
        out[0, c * T:(c + 1) * T, :] = res.results[c]["outt"].T
    return out
